# revision 30
# baseline (speedup 1.0000x reference)
"""BigBird attention (B=4, N=4096, D=1024, H=16, BS=64) on 8 TRN2 NeuronCores.

Sharding: batch (4-way) x head-group (2-way).  Core c handles batch c//2 and
heads [hg*8, hg*8+8) where hg = c%2 (d_model slice [hg*512, hg*512+512)).

Per core, fully SBUF-resident between passes (no DRAM roundtrip for q/k/v):
  pass A: QKV projections.  x.T tiles via DMA transpose; qT/kT written
          transposed ([dl, n] with 64-col wrap halo on qT), v written into a
          per-(tile,head) layout with a ones column appended so attention-AV
          matmuls emit softmax denominators for free.  Score scale folded
          into Wq/bq on the host.
  pass B: per-head BigBird attention with TRANSPOSED scores S^T = K·Q^T
          (keys on partitions, queries on free dim).  Softmax needs no max
          pass (scores bounded) and no PE transposes: exp runs on the
          scalar engine, AV matmuls contract keys directly, and the
          sliding-window masking is done structurally by restricting the
          neighbor-block AV matmuls to valid (key, query) sub-tiles.
          Local window + global-column branches are normalized separately
          (per-partition scalar ops) and summed into a natural-layout ctx;
          global-row blocks are overwritten with full attention.  ctx is
          transposed head-pair-wise via PE transposes into ctxT.
  pass C: row-parallel output projection -> partial outT [d_model, n].
Host combines: out[b] = outT(core 2b).T + outT(core 2b+1).T + bo.

The kernel is specialized (compiled) per global_indices value.
"""

import functools
import sys

import numpy as np

P = 128
BS = 64


def _ensure_path():
    try:
        import concourse.bass  # noqa: F401
    except ImportError:
        sys.path.insert(0, "/opt/trn_rl_repo")


def _build(n, dmodel, dl, g0, g1):
    """Build the per-core Bass program.

    n: sequence length, dmodel: model dim, dl: per-core head dims = hpc*64.
    g0, g1: global block indices (compile-time constants).
    """
    _ensure_path()
    from contextlib import ExitStack

    import concourse.bass as bass  # noqa: F401
    import concourse.tile as tile
    from concourse import bacc, mybir
    from concourse.masks import make_identity

    f32 = mybir.dt.float32
    bf16 = mybir.dt.bfloat16
    fp8 = mybir.dt.float8e4
    AF = mybir.ActivationFunctionType

    nch = n // 512    # 512-column chunks of the sequence
    ndc = dmodel // P  # contraction chunks for QKV proj
    njt = dl // P      # row tiles of qT/kT (head pairs)
    hpc = dl // BS     # heads per core
    nt = n // P        # 128-wide key/query tiles (2 blocks each)
    ndc2 = dl // P     # contraction chunks for out proj

    nc = bacc.Bacc(None, target_bir_lowering=False, debug=False)

    x_d = nc.dram_tensor("x", [n, dmodel], bf16, kind="ExternalInput")
    wq_d = nc.dram_tensor("wqT", [dmodel, dl], bf16, kind="ExternalInput")
    wk_d = nc.dram_tensor("wkT", [dmodel, dl], bf16, kind="ExternalInput")
    wv_d = nc.dram_tensor("wvT", [dmodel, dl], bf16, kind="ExternalInput")
    wo_d = nc.dram_tensor("woT", [dl, dmodel], bf16, kind="ExternalInput")
    bq_d = nc.dram_tensor("bq", [dl], f32, kind="ExternalInput")
    bk_d = nc.dram_tensor("bk", [dl], f32, kind="ExternalInput")
    bv_d = nc.dram_tensor("bv", [dl], f32, kind="ExternalInput")
    out_d = nc.dram_tensor("outT", [dmodel, n], f32, kind="ExternalOutput")

    with tile.TileContext(nc) as tc, ExitStack() as top:
        const = top.enter_context(tc.tile_pool(name="const", bufs=1))
        ident = const.tile([P, P], bf16)
        make_identity(nc, ident)
        onesP = const.tile([1, P], f32)
        nc.gpsimd.memset(onesP, 1.0)
        ones65 = const.tile([65, BS], bf16)
        nc.gpsimd.memset(ones65, 1.0)

        # Persistent SBUF tensors (live across passes).
        persist = top.enter_context(tc.tile_pool(name="persist", bufs=1))
        # qT with 64-col wrap halo each side: col c = 64 + seq
        qTh = persist.tile([P, njt, n + 2 * BS], bf16)
        kT = persist.tile([P, njt, n], bf16)
        # v_aug[p, kt, h, c]: seq = kt*128+p, c in [0,64) = head dim,
        # c == 64 -> 1.0 (softmax denominator column), c == 65 pad.
        v_aug = persist.tile([P, nt, hpc, 66], bf16)
        ctxT = persist.tile([P, ndc2, n], bf16)
        wo_sb = persist.tile([P, ndc2, dmodel], bf16)
        nc.gpsimd.memset(v_aug[:, :, :, 64:65], 1.0)

        # ---------------- pass A: projections ----------------
        with ExitStack() as ps:
            wpool = ps.enter_context(tc.tile_pool(name="wpool", bufs=1))
            wq_sb = wpool.tile([P, ndc, dl], bf16)
            wk_sb = wpool.tile([P, ndc, dl], bf16)
            wv_sb = wpool.tile([P, ndc, dl], bf16)
            bq_sb = wpool.tile([P, njt], f32)
            bk_sb = wpool.tile([P, njt], f32)
            bv_row = wpool.tile([1, dl], f32)
            nc.sync.dma_start(bq_sb, bq_d.rearrange("(a p) -> p a", p=P))
            nc.sync.dma_start(bk_sb, bk_d.rearrange("(a p) -> p a", p=P))
            nc.sync.dma_start(bv_row, bv_d.rearrange("(a j) -> a j", a=1))
            for dc in range(ndc):
                for w_sb_, w_d_ in ((wq_sb, wq_d), (wk_sb, wk_d), (wv_sb, wv_d)):
                    nc.sync.dma_start(
                        w_sb_[:, dc, :],
                        w_d_.rearrange("(a p) j -> p a j", p=P)[:, dc, :],
                    )

            xnpool = ps.enter_context(tc.tile_pool(name="xnpool", bufs=2))
            xn_tiles = []
            for ch0 in range(2):
                xn = xnpool.tile([P, 4, dmodel], bf16, tag="xn")
                quart = dmodel // 4
                for hh in range(4):
                    nc.scalar.dma_start(
                        xn[:, :, hh * quart : (hh + 1) * quart],
                        x_d[
                            ch0 * 512 : (ch0 + 1) * 512,
                            hh * quart : (hh + 1) * quart,
                        ].rearrange("(a p) d -> p a d", p=P),
                    )
                xn_tiles.append(xn)

            psA = ps.enter_context(tc.tile_pool(name="psA", bufs=4, space="PSUM"))

            # bv broadcast to [P, dl] via ones-matmul
            bvp = psA.tile([P, dl], f32, tag="ps_a")
            nc.tensor.matmul(bvp, onesP, bv_row, start=True, stop=True)
            bv_bc = wpool.tile([P, dl], f32)
            nc.vector.tensor_copy(bv_bc, bvp)

            xtpool = ps.enter_context(tc.tile_pool(name="xtpool", bufs=2))

            for ch in range(nch):
                n0 = ch * 512
                xT = xtpool.tile([P, ndc, 512], bf16, tag="xT")
                if ch < 2:
                    # natural (contiguous, fast) x load + PE transposes while
                    # the transpose-DMA path warms up
                    xn = xn_tiles[ch]
                    for dc in range(ndc):
                        for s in range(4):
                            ptx = psA.tile([P, P], bf16, tag="ptx", bufs=2)
                            nc.tensor.transpose(
                                ptx, xn[:, s, dc * P : (dc + 1) * P], ident
                            )
                            nc.vector.tensor_copy(
                                xT[:, dc, s * P : (s + 1) * P], ptx
                            )
                else:
                    for dc in range(ndc):
                        nc.sync.dma_start(
                            xT[:, dc, :],
                            x_d[n0 : n0 + 512, dc * P : (dc + 1) * P],
                            transpose=True,
                        )
                # qT / kT (transposed outputs, bias per-partition)
                for w_sb, b_sb, dst, off in (
                    (wq_sb, bq_sb, qTh, BS),
                    (wk_sb, bk_sb, kT, 0),
                ):
                    for jt in range(njt):
                        pp = psA.tile([P, 512], f32, tag="ps_a")
                        for dc in range(ndc):
                            nc.tensor.matmul(
                                pp,
                                w_sb[:, dc, jt * P : (jt + 1) * P],
                                xT[:, dc, :],
                                start=(dc == 0),
                                stop=(dc == ndc - 1),
                            )
                        nc.scalar.activation(
                            dst[:, jt, off + n0 : off + n0 + 512],
                            pp,
                            AF.Identity,
                            bias=b_sb[:, jt : jt + 1],
                        )
                # v (natural layout, bias broadcast along free dim), written
                # strided into the per-(tile, head) augmented layout.
                for ns in range(4):
                    kt = ch * 4 + ns
                    pp = psA.tile([P, dl], f32, tag="ps_a")
                    for dc in range(ndc):
                        nc.tensor.matmul(
                            pp,
                            xT[:, dc, ns * P : (ns + 1) * P],
                            wv_sb[:, dc, :],
                            start=(dc == 0),
                            stop=(dc == ndc - 1),
                        )
                    nc.vector.tensor_add(v_aug[:, kt, :, 0:64], pp, bv_bc)

            # qT wrap halos
            nc.vector.tensor_copy(qTh[:, :, 0:BS], qTh[:, :, n : n + BS])
            nc.vector.tensor_copy(
                qTh[:, :, n + BS : n + 2 * BS], qTh[:, :, BS : 2 * BS]
            )

        # ---------------- pass B: attention ----------------
        nc.sync.dma_start(wo_sb, wo_d.rearrange("(a p) o -> p a o", p=P))
        with ExitStack() as ps:
            hpool = ps.enter_context(tc.tile_pool(name="hpool", bufs=2))
            atpool = ps.enter_context(tc.tile_pool(name="atpool", bufs=2))
            stat = ps.enter_context(tc.tile_pool(name="stat", bufs=4))
            psB = ps.enter_context(tc.tile_pool(name="psB", bufs=2, space="PSUM"))
            ctxp = ps.enter_context(tc.tile_pool(name="ctxp", bufs=1))
            ctx_pair = ctxp.tile([P, nt, P], bf16)

            for h in range(hpc):
                p0 = (h % 2) * BS
                jt = h // 2
                c0 = p0  # ctx_pair column block for this head

                # per-head gathers: global keys/queries/values
                kTg = hpool.tile([P, P], bf16, tag="kTg")
                qg = hpool.tile([P, P], bf16, tag="qg")
                vg_aug = hpool.tile([P, 66], bf16, tag="vg")
                for gi, g in enumerate((g0, g1)):
                    nc.vector.tensor_copy(
                        kTg[p0 : p0 + BS, gi * BS : (gi + 1) * BS],
                        kT[p0 : p0 + BS, jt, g * BS : (g + 1) * BS],
                    )
                    nc.vector.tensor_copy(
                        qg[p0 : p0 + BS, gi * BS : (gi + 1) * BS],
                        qTh[p0 : p0 + BS, jt, BS + g * BS : BS + (g + 1) * BS],
                    )
                    nc.sync.dma_start(
                        vg_aug[gi * BS : (gi + 1) * BS, 0:65],
                        v_aug[(g % 2) * BS : (g % 2) * BS + BS, g // 2, h, 0:65],
                    )

                # ---- global columns: S^T_g = Kg . Q^T, exp'd, all queries ----
                aTg = hpool.tile([P, n], bf16, tag="aTg")
                for c in range(n // 512):
                    gps = psB.tile([P, 512], f32, tag="sps")
                    nc.tensor.matmul(
                        gps,
                        kTg[p0 : p0 + BS, :],
                        qTh[p0 : p0 + BS, jt, BS + c * 512 : BS + (c + 1) * 512],
                        start=True,
                        stop=True,
                    )
                    nc.scalar.activation(aTg[:, c * 512 : (c + 1) * 512], gps, AF.Exp)

                aT = atpool.tile([P, nt, 256], bf16, tag="aT")

                # ---- local sliding window scores, transposed per key tile ----
                # key tile kt (blocks 2kt, 2kt+1) scores vs queries
                # [128kt-64, 128kt+192) (wrap via qT halo).
                def score_pair(kta, ktb):
                    sps = psB.tile([P, 512], f32, tag="sps")
                    for e, kt_ in enumerate((kta, ktb)):
                        nc.tensor.matmul(
                            sps[:, e * 256 : (e + 1) * 256],
                            kT[p0 : p0 + BS, jt, kt_ * P : (kt_ + 1) * P],
                            qTh[p0 : p0 + BS, jt, kt_ * P : kt_ * P + 256],
                            start=True,
                            stop=True,
                        )
                    nc.scalar.activation(aT[:, kta : kta + 2, :], sps, AF.Exp)

                def av(t):
                    tm, tp = (t - 1) % nt, (t + 1) % nt
                    # one psum tile: cols 0:65 local branch, 65:130 global
                    pav = psB.tile([P, 130], f32, tag="pav", bufs=4)
                    # center tile: all 128 keys x all 128 queries valid
                    nc.tensor.matmul(
                        pav[:, 0:65],
                        aT[:, t, 64:192],
                        v_aug[:, t, h, 0:65],
                        start=True,
                        stop=False,
                    )
                    # left neighbor: key block 2t-1 x query block 2t only
                    nc.tensor.matmul(
                        pav[0:BS, 0:65],
                        aT[BS:P, tm, 192:256],
                        v_aug[BS:P, tm, h, 0:65],
                        start=False,
                        stop=False,
                        skip_group_check=True,
                    )
                    # right neighbor: key block 2t+2 x query block 2t+1 only
                    nc.tensor.matmul(
                        pav[BS:P, 0:65],
                        aT[0:BS, tp, 0:64],
                        v_aug[0:BS, tp, h, 0:65],
                        start=False,
                        stop=True,
                        skip_group_check=True,
                    )
                    nc.tensor.matmul(
                        pav[:, 65:130],
                        aTg[:, t * P : (t + 1) * P],
                        vg_aug[:, 0:65],
                        start=True,
                        stop=True,
                        skip_group_check=True,
                    )
                    r2 = stat.tile([P, 2], f32, tag="r2")
                    nc.vector.reciprocal(r2, pav[:, 64 : 64 + 66 : 65])
                    aa = stat.tile([P, BS], bf16, tag="aa")
                    nc.vector.tensor_scalar_mul(aa, pav[:, 0:64], r2[:, 0:1])
                    nc.vector.scalar_tensor_tensor(
                        ctx_pair[:, t, c0 : c0 + BS],
                        pav[:, 65:129],
                        r2[:, 1:2],
                        aa,
                        mybir.AluOpType.mult,
                        mybir.AluOpType.add,
                    )

                # emission order: last pair first so AV(0) has its wrap
                # neighbor; AV lags scores by one pair.
                score_pair(nt - 2, nt - 1)
                score_pair(0, 1)
                for k in range(1, nt // 2 - 1):
                    score_pair(2 * k, 2 * k + 1)
                    av(2 * k - 2)
                    av(2 * k - 1)
                for t in range(nt - 4, nt):
                    av(t)

                # ---- global rows: full attention for blocks g0, g1 ----
                prow = psB.tile([P, 65], f32, tag="prow", bufs=1)
                for c2 in range(nt // 4):
                    rps = psB.tile([P, 512], f32, tag="sps")
                    for e in range(4):
                        kt_ = c2 * 4 + e
                        nc.tensor.matmul(
                            rps[:, e * P : (e + 1) * P],
                            kT[p0 : p0 + BS, jt, kt_ * P : (kt_ + 1) * P],
                            qg[p0 : p0 + BS, :],
                            start=True,
                            stop=True,
                        )
                    aTr = hpool.tile([P, 512], bf16, tag="aTr")
                    nc.scalar.activation(aTr, rps, AF.Exp)
                    for e in range(4):
                        kt_ = c2 * 4 + e
                        nc.tensor.matmul(
                            prow,
                            aTr[:, e * P : (e + 1) * P],
                            v_aug[:, kt_, h, 0:65],
                            start=(kt_ == 0),
                            stop=(kt_ == nt - 1),
                        )
                rr = stat.tile([P, 1], f32, tag="rr")
                nc.vector.reciprocal(rr, prow[:, 64:65])
                tmp_row = stat.tile([P, BS], bf16, tag="tmp_row")
                nc.vector.tensor_scalar_mul(tmp_row, prow[:, 0:64], rr)
                for gi, g in enumerate((g0, g1)):
                    nc.sync.dma_start(
                        ctx_pair[(g % 2) * BS : (g % 2) * BS + BS, g // 2, c0 : c0 + BS],
                        tmp_row[gi * BS : (gi + 1) * BS, :],
                    )

                # ---- pair flush: transpose ctx_pair into ctxT ----
                if h % 2 == 1:
                    for t2 in range(nt // 2):
                        pT = psB.tile([P, 256], bf16, tag="pT", bufs=1)
                        nc.tensor.transpose(pT[:, 0:P], ctx_pair[:, 2 * t2, :], ident)
                        nc.tensor.transpose(
                            pT[:, P : 2 * P], ctx_pair[:, 2 * t2 + 1, :], ident
                        )
                        nc.vector.tensor_copy(
                            ctxT[:, jt, t2 * 256 : (t2 + 1) * 256], pT
                        )

        # ---------------- pass C: output projection ----------------
        with ExitStack() as ps:
            copool = ps.enter_context(tc.tile_pool(name="co", bufs=4))
            psO = ps.enter_context(tc.tile_pool(name="psO", bufs=4, space="PSUM"))
            for ot in range(dmodel // P):
                for ncc in range(n // 512):
                    pp = psO.tile([P, 512], f32, tag="pso")
                    for dc in range(ndc2):
                        nc.tensor.matmul(
                            pp,
                            wo_sb[:, dc, ot * P : (ot + 1) * P],
                            ctxT[:, dc, ncc * 512 : (ncc + 1) * 512],
                            start=(dc == 0),
                            stop=(dc == ndc2 - 1),
                        )
                    ob = copool.tile([P, 512], f32, tag="ob")
                    nc.scalar.copy(ob, pp)
                    nc.sync.dma_start(
                        out_d[ot * P : (ot + 1) * P, ncc * 512 : (ncc + 1) * 512], ob
                    )

    nc.finalize()
    return nc


@functools.lru_cache(maxsize=8)
def _get(n, dmodel, dl, g0, g1):
    return _build(n, dmodel, dl, g0, g1)


def _prepare(inputs):
    """Build (nc, in_maps, meta) for the SPMD run from full unsharded inputs."""
    x = np.asarray(inputs["x"], np.float32)
    Wq = np.asarray(inputs["Wq"], np.float32)
    Wk = np.asarray(inputs["Wk"], np.float32)
    Wv = np.asarray(inputs["Wv"], np.float32)
    Wo = np.asarray(inputs["Wo"], np.float32)
    bq = np.asarray(inputs["bq"], np.float32)
    bk = np.asarray(inputs["bk"], np.float32)
    bv = np.asarray(inputs["bv"], np.float32)
    bo = np.asarray(inputs["bo"], np.float32)
    gi = np.asarray(inputs["global_indices"]).astype(np.int64)
    g0, g1 = int(gi[0]), int(gi[1])

    b_, n_, d_ = x.shape
    dl = d_ // 2
    scale = 1.0 / np.sqrt(np.float32(64.0)).astype(np.float32)

    nc = _get(n_, d_, dl, g0, g1)

    import ml_dtypes

    bf = ml_dtypes.bfloat16
    in_maps = []
    for c in range(8):
        b, hg = divmod(c, 2)
        S = slice(hg * dl, (hg + 1) * dl)
        in_maps.append(
            {
                "x": np.ascontiguousarray(x[b]).astype(bf),
                "wqT": np.ascontiguousarray((Wq[S, :] * scale).T).astype(bf),
                "wkT": np.ascontiguousarray(Wk[S, :].T).astype(bf),
                "wvT": np.ascontiguousarray(Wv[S, :].T).astype(bf),
                "woT": np.ascontiguousarray(Wo[:, S].T).astype(bf),
                "bq": np.ascontiguousarray(bq[S] * scale),
                "bk": np.ascontiguousarray(bk[S]),
                "bv": np.ascontiguousarray(bv[S]),
            }
        )

    return nc, in_maps, (b_, n_, d_, bo)


def _combine(res, meta):
    b_, n_, d_, bo = meta
    out = np.empty((b_, n_, d_), np.float32)
    for b in range(b_):
        out[b] = res[2 * b]["outT"].T + res[2 * b + 1]["outT"].T + bo[None, :]
    return out


def kernel(**inputs):
    _ensure_path()
    from concourse.bass_utils import run_bass_kernel_spmd

    nc, in_maps, meta = _prepare(inputs)
    res = run_bass_kernel_spmd(nc, in_maps, list(range(8))).results
    return _combine(res, meta)


# revision 31
# speedup vs baseline: 1.0305x; 1.0305x over previous
"""BigBird attention (B=4, N=4096, D=1024, H=16, BS=64) on 8 TRN2 NeuronCores.

Sharding: batch (4-way) x head-group (2-way).  Core c handles batch c//2 and
heads [hg*8, hg*8+8) where hg = c%2 (d_model slice [hg*512, hg*512+512)).

Per core, fully SBUF-resident between passes (no DRAM roundtrip for q/k/v):
  pass A: QKV projections.  x.T tiles via DMA transpose; qT/kT written
          transposed ([dl, n] with 64-col wrap halo on qT), v written into a
          per-(tile,head) layout with a ones column appended so attention-AV
          matmuls emit softmax denominators for free.  Score scale folded
          into Wq/bq on the host.
  pass B: per-head BigBird attention with TRANSPOSED scores S^T = K·Q^T
          (keys on partitions, queries on free dim).  Softmax needs no max
          pass (scores bounded) and no PE transposes: exp runs on the
          scalar engine, AV matmuls contract keys directly, and the
          sliding-window masking is done structurally by restricting the
          neighbor-block AV matmuls to valid (key, query) sub-tiles.
          Local window + global-column branches are normalized separately
          (per-partition scalar ops) and summed into a natural-layout ctx;
          global-row blocks are overwritten with full attention.  ctx is
          transposed head-pair-wise via PE transposes into ctxT.
  pass C: row-parallel output projection -> partial outT [d_model, n].
Host combines: out[b] = outT(core 2b).T + outT(core 2b+1).T + bo.

The kernel is specialized (compiled) per global_indices value.
"""

import functools
import sys

import numpy as np

P = 128
BS = 64


def _ensure_path():
    try:
        import concourse.bass  # noqa: F401
    except ImportError:
        sys.path.insert(0, "/opt/trn_rl_repo")


def _build(n, dmodel, dl, g0, g1):
    """Build the per-core Bass program.

    n: sequence length, dmodel: model dim, dl: per-core head dims = hpc*64.
    g0, g1: global block indices (compile-time constants).
    """
    _ensure_path()
    from contextlib import ExitStack

    import concourse.bass as bass  # noqa: F401
    import concourse.tile as tile
    from concourse import bacc, mybir
    from concourse.masks import make_identity

    f32 = mybir.dt.float32
    bf16 = mybir.dt.bfloat16
    fp8 = mybir.dt.float8e4
    AF = mybir.ActivationFunctionType

    nch = n // 512    # 512-column chunks of the sequence
    ndc = dmodel // P  # contraction chunks for QKV proj
    njt = dl // P      # row tiles of qT/kT (head pairs)
    hpc = dl // BS     # heads per core
    nt = n // P        # 128-wide key/query tiles (2 blocks each)
    ndc2 = dl // P     # contraction chunks for out proj

    nc = bacc.Bacc(None, target_bir_lowering=False, debug=False)

    x_d = nc.dram_tensor("x", [n, dmodel], bf16, kind="ExternalInput")
    wq_d = nc.dram_tensor("wqT", [dmodel, dl], bf16, kind="ExternalInput")
    wk_d = nc.dram_tensor("wkT", [dmodel, dl], bf16, kind="ExternalInput")
    wv_d = nc.dram_tensor("wvT", [dmodel, dl], bf16, kind="ExternalInput")
    wo_d = nc.dram_tensor("woT", [dl, dmodel], bf16, kind="ExternalInput")
    bq_d = nc.dram_tensor("bq", [dl], f32, kind="ExternalInput")
    bk_d = nc.dram_tensor("bk", [dl], f32, kind="ExternalInput")
    bv_d = nc.dram_tensor("bv", [dl], f32, kind="ExternalInput")
    out_d = nc.dram_tensor("outT", [dmodel, n], f32, kind="ExternalOutput")

    with tile.TileContext(nc) as tc, ExitStack() as top:
        const = top.enter_context(tc.tile_pool(name="const", bufs=1))
        ident = const.tile([P, P], bf16)
        make_identity(nc, ident)
        onesP = const.tile([1, P], f32)
        nc.gpsimd.memset(onesP, 1.0)
        ones65 = const.tile([65, BS], bf16)
        nc.gpsimd.memset(ones65, 1.0)

        # Persistent SBUF tensors (live across passes).
        persist = top.enter_context(tc.tile_pool(name="persist", bufs=1))
        # qT with 64-col wrap halo each side: col c = 64 + seq
        qTh = persist.tile([P, njt, n + 2 * BS], bf16)
        kT = persist.tile([P, njt, n], bf16)
        # v_aug[p, kt, h, c]: seq = kt*128+p, c in [0,64) = head dim,
        # c == 64 -> 1.0 (softmax denominator column), c == 65 pad.
        v_aug = persist.tile([P, nt, hpc, 66], bf16)
        ctxT = persist.tile([P, ndc2, n], bf16)
        wo_sb = persist.tile([P, ndc2, dmodel], bf16)
        nc.gpsimd.memset(v_aug[:, :, :, 64:65], 1.0)

        # ---------------- pass A: projections ----------------
        with ExitStack() as ps:
            wpool = ps.enter_context(tc.tile_pool(name="wpool", bufs=1))
            wq_sb = wpool.tile([P, ndc, dl], bf16)
            wk_sb = wpool.tile([P, ndc, dl], bf16)
            wv_sb = wpool.tile([P, ndc, dl], bf16)
            bq_sb = wpool.tile([P, njt], f32)
            bk_sb = wpool.tile([P, njt], f32)
            bv_row = wpool.tile([1, dl], f32)
            nc.sync.dma_start(bq_sb, bq_d.rearrange("(a p) -> p a", p=P))
            nc.sync.dma_start(bk_sb, bk_d.rearrange("(a p) -> p a", p=P))
            nc.sync.dma_start(bv_row, bv_d.rearrange("(a j) -> a j", a=1))
            for dc in range(ndc):
                for w_sb_, w_d_ in ((wq_sb, wq_d), (wk_sb, wk_d), (wv_sb, wv_d)):
                    nc.sync.dma_start(
                        w_sb_[:, dc, :],
                        w_d_.rearrange("(a p) j -> p a j", p=P)[:, dc, :],
                    )

            xnpool = ps.enter_context(tc.tile_pool(name="xnpool", bufs=2))
            xn_tiles = []
            for ch0 in range(2):
                xn = xnpool.tile([P, 4, dmodel], bf16, tag="xn")
                quart = dmodel // 4
                for hh in range(4):
                    nc.scalar.dma_start(
                        xn[:, :, hh * quart : (hh + 1) * quart],
                        x_d[
                            ch0 * 512 : (ch0 + 1) * 512,
                            hh * quart : (hh + 1) * quart,
                        ].rearrange("(a p) d -> p a d", p=P),
                    )
                xn_tiles.append(xn)

            psA = ps.enter_context(tc.tile_pool(name="psA", bufs=4, space="PSUM"))

            # bv broadcast to [P, dl] via ones-matmul
            bvp = psA.tile([P, dl], f32, tag="ps_a")
            nc.tensor.matmul(bvp, onesP, bv_row, start=True, stop=True)
            bv_bc = wpool.tile([P, dl], f32)
            nc.vector.tensor_copy(bv_bc, bvp)

            xtpool = ps.enter_context(tc.tile_pool(name="xtpool", bufs=2))

            for ch in range(nch):
                n0 = ch * 512
                xT = xtpool.tile([P, ndc, 512], bf16, tag="xT")
                if ch < 2:
                    # natural (contiguous, fast) x load + PE transposes while
                    # the transpose-DMA path warms up
                    xn = xn_tiles[ch]
                    for dc in range(ndc):
                        for s in range(4):
                            ptx = psA.tile([P, P], bf16, tag="ptx", bufs=2)
                            nc.tensor.transpose(
                                ptx, xn[:, s, dc * P : (dc + 1) * P], ident
                            )
                            nc.vector.tensor_copy(
                                xT[:, dc, s * P : (s + 1) * P], ptx
                            )
                else:
                    for dc in range(ndc):
                        nc.sync.dma_start(
                            xT[:, dc, :],
                            x_d[n0 : n0 + 512, dc * P : (dc + 1) * P],
                            transpose=True,
                        )
                # qT / kT (transposed outputs, bias per-partition)
                for w_sb, b_sb, dst, off in (
                    (wq_sb, bq_sb, qTh, BS),
                    (wk_sb, bk_sb, kT, 0),
                ):
                    for jt in range(njt):
                        pp = psA.tile([P, 512], f32, tag="ps_a")
                        for dc in range(ndc):
                            nc.tensor.matmul(
                                pp,
                                w_sb[:, dc, jt * P : (jt + 1) * P],
                                xT[:, dc, :],
                                start=(dc == 0),
                                stop=(dc == ndc - 1),
                            )
                        nc.scalar.activation(
                            dst[:, jt, off + n0 : off + n0 + 512],
                            pp,
                            AF.Identity,
                            bias=b_sb[:, jt : jt + 1],
                        )
                # v (natural layout, bias broadcast along free dim), written
                # strided into the per-(tile, head) augmented layout.
                for ns in range(4):
                    kt = ch * 4 + ns
                    pp = psA.tile([P, dl], f32, tag="ps_a")
                    for dc in range(ndc):
                        nc.tensor.matmul(
                            pp,
                            xT[:, dc, ns * P : (ns + 1) * P],
                            wv_sb[:, dc, :],
                            start=(dc == 0),
                            stop=(dc == ndc - 1),
                        )
                    nc.vector.tensor_add(v_aug[:, kt, :, 0:64], pp, bv_bc)

            # qT wrap halos
            nc.vector.tensor_copy(qTh[:, :, 0:BS], qTh[:, :, n : n + BS])
            nc.vector.tensor_copy(
                qTh[:, :, n + BS : n + 2 * BS], qTh[:, :, BS : 2 * BS]
            )

        # ---------------- pass B: attention ----------------
        nc.sync.dma_start(wo_sb, wo_d.rearrange("(a p) o -> p a o", p=P))
        with ExitStack() as ps:
            hpool = ps.enter_context(tc.tile_pool(name="hpool", bufs=2))
            atpool = ps.enter_context(tc.tile_pool(name="atpool", bufs=2))
            stat = ps.enter_context(tc.tile_pool(name="stat", bufs=4))
            psB = ps.enter_context(tc.tile_pool(name="psB", bufs=2, space="PSUM"))
            ctxp = ps.enter_context(tc.tile_pool(name="ctxp", bufs=1))
            ctx_pair = ctxp.tile([P, nt, P], bf16)

            for h in range(hpc):
                p0 = (h % 2) * BS
                jt = h // 2
                c0 = p0  # ctx_pair column block for this head

                # per-head gathers: global keys/queries/values
                kTg = hpool.tile([P, P], bf16, tag="kTg")
                qg = hpool.tile([P, P], bf16, tag="qg")
                vg_aug = hpool.tile([P, 66], bf16, tag="vg")
                for gi, g in enumerate((g0, g1)):
                    nc.vector.tensor_copy(
                        kTg[p0 : p0 + BS, gi * BS : (gi + 1) * BS],
                        kT[p0 : p0 + BS, jt, g * BS : (g + 1) * BS],
                    )
                    nc.vector.tensor_copy(
                        qg[p0 : p0 + BS, gi * BS : (gi + 1) * BS],
                        qTh[p0 : p0 + BS, jt, BS + g * BS : BS + (g + 1) * BS],
                    )
                    nc.sync.dma_start(
                        vg_aug[gi * BS : (gi + 1) * BS, 0:65],
                        v_aug[(g % 2) * BS : (g % 2) * BS + BS, g // 2, h, 0:65],
                    )

                # ---- global columns: S^T_g = Kg . Q^T, exp'd, all queries ----
                aTg = hpool.tile([P, n], bf16, tag="aTg")
                for c in range(n // 512):
                    gps = psB.tile([P, 512], f32, tag="sps")
                    nc.tensor.matmul(
                        gps,
                        kTg[p0 : p0 + BS, :],
                        qTh[p0 : p0 + BS, jt, BS + c * 512 : BS + (c + 1) * 512],
                        start=True,
                        stop=True,
                    )
                    nc.scalar.activation(aTg[:, c * 512 : (c + 1) * 512], gps, AF.Exp)

                aT = atpool.tile([P, nt, 256], bf16, tag="aT")

                # ---- local sliding window scores, transposed per key tile ----
                # key tile kt (blocks 2kt, 2kt+1) scores vs queries
                # [128kt-64, 128kt+192) (wrap via qT halo).
                def score_pair(kta, ktb):
                    sps = psB.tile([P, 512], f32, tag="sps")
                    for e, kt_ in enumerate((kta, ktb)):
                        nc.tensor.matmul(
                            sps[:, e * 256 : (e + 1) * 256],
                            kT[p0 : p0 + BS, jt, kt_ * P : (kt_ + 1) * P],
                            qTh[p0 : p0 + BS, jt, kt_ * P : kt_ * P + 256],
                            start=True,
                            stop=True,
                        )
                    nc.scalar.activation(aT[:, kta : kta + 2, :], sps, AF.Exp)

                def av(t):
                    tm, tp = (t - 1) % nt, (t + 1) % nt
                    # one psum tile: cols 0:65 local branch, 65:130 global
                    pav = psB.tile([P, 130], f32, tag="pav", bufs=3)
                    # center tile: all 128 keys x all 128 queries valid
                    nc.tensor.matmul(
                        pav[:, 0:65],
                        aT[:, t, 64:192],
                        v_aug[:, t, h, 0:65],
                        start=True,
                        stop=False,
                    )
                    # left neighbor: key block 2t-1 x query block 2t only
                    nc.tensor.matmul(
                        pav[0:BS, 0:65],
                        aT[BS:P, tm, 192:256],
                        v_aug[BS:P, tm, h, 0:65],
                        start=False,
                        stop=False,
                        skip_group_check=True,
                    )
                    # right neighbor: key block 2t+2 x query block 2t+1 only
                    nc.tensor.matmul(
                        pav[BS:P, 0:65],
                        aT[0:BS, tp, 0:64],
                        v_aug[0:BS, tp, h, 0:65],
                        start=False,
                        stop=True,
                        skip_group_check=True,
                    )
                    nc.tensor.matmul(
                        pav[:, 65:130],
                        aTg[:, t * P : (t + 1) * P],
                        vg_aug[:, 0:65],
                        start=True,
                        stop=True,
                        skip_group_check=True,
                    )
                    r2 = stat.tile([P, 2], f32, tag="r2")
                    nc.vector.reciprocal(r2, pav[:, 64 : 64 + 66 : 65])
                    aa = stat.tile([P, BS], bf16, tag="aa")
                    nc.vector.tensor_scalar_mul(aa, pav[:, 0:64], r2[:, 0:1])
                    nc.vector.scalar_tensor_tensor(
                        ctx_pair[:, t, c0 : c0 + BS],
                        pav[:, 65:129],
                        r2[:, 1:2],
                        aa,
                        mybir.AluOpType.mult,
                        mybir.AluOpType.add,
                    )

                # emission order: last pair first so AV(0) has its wrap
                # neighbor; AV lags scores by one pair.
                score_pair(nt - 2, nt - 1)
                score_pair(0, 1)
                for k in range(1, nt // 2 - 1):
                    score_pair(2 * k, 2 * k + 1)
                    av(2 * k - 2)
                    av(2 * k - 1)
                for t in range(nt - 4, nt):
                    av(t)

                # ---- global rows: full attention for blocks g0, g1 ----
                prow = psB.tile([P, 65], f32, tag="prow", bufs=1)
                for c2 in range(nt // 4):
                    rps = psB.tile([P, 512], f32, tag="sps")
                    for e in range(4):
                        kt_ = c2 * 4 + e
                        nc.tensor.matmul(
                            rps[:, e * P : (e + 1) * P],
                            kT[p0 : p0 + BS, jt, kt_ * P : (kt_ + 1) * P],
                            qg[p0 : p0 + BS, :],
                            start=True,
                            stop=True,
                        )
                    aTr = hpool.tile([P, 512], bf16, tag="aTr")
                    nc.scalar.activation(aTr, rps, AF.Exp)
                    for e in range(4):
                        kt_ = c2 * 4 + e
                        nc.tensor.matmul(
                            prow,
                            aTr[:, e * P : (e + 1) * P],
                            v_aug[:, kt_, h, 0:65],
                            start=(kt_ == 0),
                            stop=(kt_ == nt - 1),
                        )
                rr = stat.tile([P, 1], f32, tag="rr")
                nc.vector.reciprocal(rr, prow[:, 64:65])
                tmp_row = stat.tile([P, BS], bf16, tag="tmp_row")
                nc.vector.tensor_scalar_mul(tmp_row, prow[:, 0:64], rr)
                for gi, g in enumerate((g0, g1)):
                    nc.sync.dma_start(
                        ctx_pair[(g % 2) * BS : (g % 2) * BS + BS, g // 2, c0 : c0 + BS],
                        tmp_row[gi * BS : (gi + 1) * BS, :],
                    )

                # ---- pair flush: transpose ctx_pair into ctxT ----
                if h % 2 == 1:
                    for t2 in range(nt // 2):
                        pT = psB.tile([P, 256], bf16, tag="pT")
                        nc.tensor.transpose(pT[:, 0:P], ctx_pair[:, 2 * t2, :], ident)
                        nc.tensor.transpose(
                            pT[:, P : 2 * P], ctx_pair[:, 2 * t2 + 1, :], ident
                        )
                        nc.vector.tensor_copy(
                            ctxT[:, jt, t2 * 256 : (t2 + 1) * 256], pT
                        )

        # ---------------- pass C: output projection ----------------
        with ExitStack() as ps:
            copool = ps.enter_context(tc.tile_pool(name="co", bufs=4))
            psO = ps.enter_context(tc.tile_pool(name="psO", bufs=4, space="PSUM"))
            for ot in range(dmodel // P):
                for ncc in range(n // 512):
                    pp = psO.tile([P, 512], f32, tag="pso")
                    for dc in range(ndc2):
                        nc.tensor.matmul(
                            pp,
                            wo_sb[:, dc, ot * P : (ot + 1) * P],
                            ctxT[:, dc, ncc * 512 : (ncc + 1) * 512],
                            start=(dc == 0),
                            stop=(dc == ndc2 - 1),
                        )
                    ob = copool.tile([P, 512], f32, tag="ob")
                    nc.scalar.copy(ob, pp)
                    nc.sync.dma_start(
                        out_d[ot * P : (ot + 1) * P, ncc * 512 : (ncc + 1) * 512], ob
                    )

    nc.finalize()
    return nc


@functools.lru_cache(maxsize=8)
def _get(n, dmodel, dl, g0, g1):
    return _build(n, dmodel, dl, g0, g1)


def _prepare(inputs):
    """Build (nc, in_maps, meta) for the SPMD run from full unsharded inputs."""
    x = np.asarray(inputs["x"], np.float32)
    Wq = np.asarray(inputs["Wq"], np.float32)
    Wk = np.asarray(inputs["Wk"], np.float32)
    Wv = np.asarray(inputs["Wv"], np.float32)
    Wo = np.asarray(inputs["Wo"], np.float32)
    bq = np.asarray(inputs["bq"], np.float32)
    bk = np.asarray(inputs["bk"], np.float32)
    bv = np.asarray(inputs["bv"], np.float32)
    bo = np.asarray(inputs["bo"], np.float32)
    gi = np.asarray(inputs["global_indices"]).astype(np.int64)
    g0, g1 = int(gi[0]), int(gi[1])

    b_, n_, d_ = x.shape
    dl = d_ // 2
    scale = 1.0 / np.sqrt(np.float32(64.0)).astype(np.float32)

    nc = _get(n_, d_, dl, g0, g1)

    import ml_dtypes

    bf = ml_dtypes.bfloat16
    in_maps = []
    for c in range(8):
        b, hg = divmod(c, 2)
        S = slice(hg * dl, (hg + 1) * dl)
        in_maps.append(
            {
                "x": np.ascontiguousarray(x[b]).astype(bf),
                "wqT": np.ascontiguousarray((Wq[S, :] * scale).T).astype(bf),
                "wkT": np.ascontiguousarray(Wk[S, :].T).astype(bf),
                "wvT": np.ascontiguousarray(Wv[S, :].T).astype(bf),
                "woT": np.ascontiguousarray(Wo[:, S].T).astype(bf),
                "bq": np.ascontiguousarray(bq[S] * scale),
                "bk": np.ascontiguousarray(bk[S]),
                "bv": np.ascontiguousarray(bv[S]),
            }
        )

    return nc, in_maps, (b_, n_, d_, bo)


def _combine(res, meta):
    b_, n_, d_, bo = meta
    out = np.empty((b_, n_, d_), np.float32)
    for b in range(b_):
        out[b] = res[2 * b]["outT"].T + res[2 * b + 1]["outT"].T + bo[None, :]
    return out


def kernel(**inputs):
    _ensure_path()
    from concourse.bass_utils import run_bass_kernel_spmd

    nc, in_maps, meta = _prepare(inputs)
    res = run_bass_kernel_spmd(nc, in_maps, list(range(8))).results
    return _combine(res, meta)
